# revision 12
# baseline (speedup 1.0000x reference)
"""Brownian-bridge criterion loss on 8 Trainium2 NeuronCores.

Two launches (down from three), bf16 matmul operands, and a cross-core
AllGather collective for the tiny top-8 exchange:

  Host (indexing only): sort sequences by bridge pivot; core k owns the
  200 sorted cur sequences [200k, 200k+200) plus 200 other sequences.
  Rows are laid out t-major: cur rows t*200+s (t=0..15), oth rows
  3200+(t-1)*200+s (t=1..14 only -- head/tail of `other` are never
  used). 6000 rows padded to 6016 = 47 partition tiles.

  Launch A (per core): project+bias its 6016 rows with W,b via three
  accumulating matmuls per tile (bias folded in as a K=1 ones-row
  matmul), square+row-sum on DVE straight out of PSUM, batched
  sqrt/reciprocal, scale-copy to bf16 on ACT.  Pool rows (t=1..14 of
  both cur and oth) go to DRAM as bf16.  Epilogue reads g0/g2 directly
  from SBUF slices (t-major makes them contiguous), gathers g1 by pivot
  via indirect DMA, and emits per-sequence c0, c1, s(self-dist), score
  as f32.  No Exp/Ln here -> single ACT table (sqrt).

  Host reshard (indexing only): scatter a^T into the pivot-grouped
  padded A matrix (256 x 3584 slots, group capacity 256); build each
  core's negative-pool slice [256, 14, 400] from its own emb; pack the
  per-sequence scalars into slot layout [128, 28, 4] with pad values
  (c0=c1=0, s=-1e4, score=+1e4) that contribute exactly zero.

  Launch B (per core): 28 M-tiles x 2 matmuls -> cross [128,400] PSUM,
  Max8 -> per-core top-8 per slot; AllGather (115KB) of the top-8
  across all 8 cores; then every core (replicated) computes top-8 of
  the 64 candidates, the affine dist = c1*cross + c0 (monotone, so
  top-k commutes), numer = exp(s), the self-exclusion identity
  deno = numer + sum_{j<=5} exp(v_j) - exp(max(v_5, s)), the softplus
  head-tail term, and the two means via a ones-matmul.

The kernel structure is value-independent: bridge contents only change
index/input tensors, never shapes or instruction streams.
"""

import sys

sys.path.insert(0, "/opt/trn_rl_repo")

import numpy as np
import ml_dtypes

import concourse.bacc as bacc
import concourse.bass as bass
import concourse.mybir as mybir
import concourse.tile as tile
from concourse.bass_utils import run_bass_kernel_spmd

F32 = mybir.dt.float32
BF16 = mybir.dt.bfloat16
I32 = mybir.dt.int32
AF = mybir.ActivationFunctionType
OP = mybir.AluOpType
NPBF = ml_dtypes.bfloat16

BS, T, Q, HID, PROJ = 16, 16, 100, 256, 256
NSEQ = BS * Q              # 1600 positive sequences
NCORES = 8
SPC = NSEQ // NCORES       # 200 cur sequences per core
CUR_ROWS = T * SPC         # 3200 t-major cur rows
OTH_ROWS = (T - 2) * SPC   # 2800 t-major oth rows (t=1..14 only)
ROWS = CUR_ROWS + OTH_ROWS  # 6000
NT = 47                    # partition tiles (47*128 = 6016)
RPAD = NT * 128
POOL_CUR = (T - 2) * SPC   # 2800 pool rows from cur (t=1..14)
POOL_ROWS = 2 * POOL_CUR   # 5600 emb rows in DRAM
NG = T - 2                 # 14 pivot groups
GCAP = 256                 # padded slots per group
SLOTS = NG * GCAP          # 3584
MT = SLOTS // 128          # 28 M-tiles in launch B
NCOL = 2 * SPC             # 400 negative-pool columns per core
DELTA = 0.3
GRP = 6                    # tiles per rsqrt batch in launch A


def _build_la():
    nc = bacc.Bacc("TRN2", target_bir_lowering=False, debug=False,
                   num_devices=NCORES)
    xt_in = nc.declare_dram_parameter("xt_in", [HID, RPAD], BF16,
                                      isOutput=False)
    w_in = nc.declare_dram_parameter("w_in", [HID, PROJ], BF16, isOutput=False)
    b_in = nc.declare_dram_parameter("b_in", [1, PROJ], BF16, isOutput=False)
    br_in = nc.declare_dram_parameter("br_in", [SPC, 3], I32, isOutput=False)
    g1idx = nc.declare_dram_parameter("g1idx", [SPC, 1], I32, isOutput=False)

    emb = nc.declare_dram_parameter("emb", [POOL_ROWS, PROJ], BF16,
                                    isOutput=True)
    a_out = nc.declare_dram_parameter("a_out", [SPC, PROJ], BF16,
                                      isOutput=True)
    sc_out = nc.declare_dram_parameter("sc_out", [SPC, 4], F32, isOutput=True)
    # sc_out cols: 0=c0, 1=c1, 2=s(self dist), 3=score

    with tile.TileContext(nc) as tc:
        with (
            tc.tile_pool(name="singles", bufs=1) as singles,
            tc.tile_pool(name="work", bufs=8) as work,
            tc.tile_pool(name="ework", bufs=2) as ework,
            tc.tile_pool(name="psum", bufs=8, space="PSUM") as psum_pool,
        ):
            # persistent operands, split across the two HWDGE queues;
            # xt halves land in matmul order
            xt_sb = []
            engs = (nc.sync, nc.scalar)
            for kt in range(2):
                t_x = singles.tile([128, RPAD], BF16, tag=f"xt{kt}")
                half = RPAD // 2
                engs[kt].dma_start(out=t_x[:, :half],
                                   in_=xt_in[kt * 128:(kt + 1) * 128, :half])
                engs[kt].dma_start(out=t_x[:, half:],
                                   in_=xt_in[kt * 128:(kt + 1) * 128, half:])
                xt_sb.append(t_x)
            w_sb = []
            for kt in range(2):
                t_w = singles.tile([128, PROJ], BF16, tag=f"w{kt}")
                engs[kt].dma_start(out=t_w, in_=w_in[kt * 128:(kt + 1) * 128, :])
                w_sb.append(t_w)
            b_sb = singles.tile([1, PROJ], BF16, tag="bias")
            nc.scalar.dma_start(out=b_sb, in_=b_in[:, :])
            ones1 = singles.tile([1, 128], BF16, tag="ones1")
            nc.gpsimd.memset(ones1, 1.0)

            ss_all = singles.tile([128, NT + 1], F32, tag="ss")
            sn_all = singles.tile([128, NT + 1], F32, tag="sn")
            rs_all = singles.tile([128, NT + 1], F32, tag="rs")
            ys_all = singles.tile([128, NT, PROJ], BF16, tag="ys")

            # emb DRAM row for local row r (t-major):
            #   cur pool rows 200..2999   -> r - 200
            #   oth pool rows 3200..5999  -> r - 400
            def emb_ranges(m):
                lo, hi = m * 128, (m + 1) * 128
                out = []
                a, b2 = max(lo, SPC), min(hi, CUR_ROWS - SPC)
                if a < b2:
                    out.append((a - lo, b2 - lo, a - SPC))
                a, b2 = max(lo, CUR_ROWS), min(hi, ROWS)
                if a < b2:
                    out.append((a - lo, b2 - lo, a - 2 * SPC))
                return out

            xs_live = {}
            out_eng = 0
            for m in range(NT):
                ps = psum_pool.tile([128, PROJ], F32)
                nc.tensor.matmul(out=ps, lhsT=ones1, rhs=b_sb,
                                 start=True, stop=False)
                for kt in range(2):
                    nc.tensor.matmul(
                        out=ps,
                        lhsT=xt_sb[kt][:, m * 128:(m + 1) * 128],
                        rhs=w_sb[kt],
                        start=False,
                        stop=(kt == 1),
                    )
                # ACT evacuates PSUM (single read; frees the bank fast),
                # DVE squares+row-sums from SBUF
                xs = work.tile([128, PROJ], F32, tag="xs")
                nc.scalar.activation(out=xs, in_=ps, func=AF.Copy)
                sqt = work.tile([128, PROJ], F32, tag="sq")
                nc.vector.scalar_tensor_tensor(
                    out=sqt, in0=xs, scalar=1.0, in1=xs,
                    op0=OP.mult, op1=OP.mult,
                    accum_out=ss_all[:, m:m + 1])
                xs_live[m] = xs

                if m % GRP == GRP - 1 or m == NT - 1:
                    g0 = (m // GRP) * GRP
                    sl = slice(g0, m + 1)
                    nc.scalar.activation(out=sn_all[:, sl], in_=ss_all[:, sl],
                                         func=AF.Sqrt)
                    nc.vector.reciprocal(out=rs_all[:, sl], in_=sn_all[:, sl])
                    for mm in range(g0, m + 1):
                        nc.vector.tensor_scalar(
                            out=ys_all[:, mm, :], in0=xs_live.pop(mm),
                            scalar1=rs_all[:, mm:mm + 1], scalar2=None,
                            op0=OP.mult)
                        for (p0, p1, dst) in emb_ranges(mm):
                            eng = nc.gpsimd if out_eng % 2 == 0 else nc.sync
                            out_eng += 1
                            eng.dma_start(
                                out=emb[dst:dst + (p1 - p0), :],
                                in_=ys_all[p0:p1, mm, :])

            # ---- epilogue: per-sequence scalars -------------------------
            # t-major row positions: g0 = rows 0..199 (tiles 0, 1),
            # g2 = rows 3000..3199 (tile 23 p56.., tile 24).
            g2a = singles.tile([128, PROJ], BF16, tag="g2a")
            g2b = singles.tile([128, PROJ], BF16, tag="g2b")
            nc.sync.dma_start(out=g2a[0:72], in_=ys_all[56:128, 23, :])
            nc.sync.dma_start(out=g2a[72:128], in_=ys_all[0:56, 24, :])
            nc.sync.dma_start(out=g2b[0:72], in_=ys_all[56:128, 24, :])

            for ti, (s0, psz) in enumerate(((0, 128), (128, 72))):
                g0t = (ys_all[:, 0, :] if ti == 0 else ys_all[0:72, 1, :])
                g2t = (g2a if ti == 0 else g2b)[:psz]
                idx = ework.tile([128, 1], I32, tag="idx")
                nc.scalar.dma_start(out=idx[:psz], in_=g1idx[s0:s0 + psz, :])
                g1t = ework.tile([128, PROJ], BF16, tag="g1")
                nc.gpsimd.indirect_dma_start(
                    out=g1t[:psz],
                    out_offset=None,
                    in_=emb[:, :],
                    in_offset=bass.IndirectOffsetOnAxis(ap=idx[:psz, :1],
                                                        axis=0),
                )
                bi = ework.tile([128, 3], I32, tag="bi")
                nc.scalar.dma_start(out=bi[:psz], in_=br_in[s0:s0 + psz, :])
                bf = ework.tile([128, 3], F32, tag="bf")
                nc.vector.tensor_copy(out=bf[:psz], in_=bi[:psz])
                bh, bp, bt = bf[:psz, 0:1], bf[:psz, 1:2], bf[:psz, 2:3]

                def tt(o, i0, i1, op):
                    nc.vector.tensor_tensor(out=o, in0=i0, in1=i1, op=op)

                sc = ework.tile([128, 16], F32, tag="sc")
                c0 = sc[:psz, 0:1]
                c1 = sc[:psz, 1:2]
                s_sd = sc[:psz, 2:3]
                score = sc[:psz, 3:4]
                alpha = sc[:psz, 4:5]
                d2 = sc[:psz, 5:6]
                sig = sc[:psz, 6:7]
                q = sc[:psz, 7:8]
                aa = sc[:psz, 8:9]
                tmp = sc[:psz, 9:10]
                oma = sc[:psz, 10:11]

                tt(alpha, bp, bh, OP.subtract)          # bp - bh
                tt(d2, bt, bh, OP.subtract)             # bt - bh
                nc.vector.reciprocal(out=d2, in_=d2)
                tt(alpha, alpha, d2, OP.mult)           # alpha
                tt(sig, bt, bp, OP.subtract)            # bt - bp
                tt(sig, alpha, sig, OP.mult)            # sigma
                tt(sig, sig, sig, OP.mult)              # sigma^2
                nc.vector.reciprocal(out=c1, in_=sig)   # c1 = 1/sigma^2

                a_t = ework.tile([128, PROJ], BF16, tag="a")
                prod = ework.tile([128, PROJ], F32, tag="prod")
                nc.vector.tensor_scalar(out=oma, in0=alpha, scalar1=-1.0,
                                        scalar2=1.0, op0=OP.mult, op1=OP.add)
                nc.vector.tensor_scalar(out=prod[:psz], in0=g0t, scalar1=oma,
                                        scalar2=None, op0=OP.mult)
                nc.vector.scalar_tensor_tensor(
                    out=a_t[:psz], in0=g2t, scalar=alpha, in1=prod[:psz],
                    op0=OP.mult, op1=OP.add)

                # q = a.g1 ; aa = a.a ; score = g0.g2
                nc.vector.scalar_tensor_tensor(
                    out=prod[:psz], in0=a_t[:psz], scalar=1.0, in1=g1t[:psz],
                    op0=OP.mult, op1=OP.mult, accum_out=q)
                nc.vector.scalar_tensor_tensor(
                    out=prod[:psz], in0=a_t[:psz], scalar=1.0, in1=a_t[:psz],
                    op0=OP.mult, op1=OP.mult, accum_out=aa)
                nc.vector.scalar_tensor_tensor(
                    out=prod[:psz], in0=g0t, scalar=1.0, in1=g2t,
                    op0=OP.mult, op1=OP.mult, accum_out=score)

                # s = -(1 - 2q + aa) / (2 sigma^2)
                nc.vector.tensor_scalar(out=tmp, in0=q, scalar1=-2.0,
                                        scalar2=1.0, op0=OP.mult, op1=OP.add)
                tt(tmp, tmp, aa, OP.add)
                nc.vector.tensor_scalar(out=s_sd, in0=tmp, scalar1=c1,
                                        scalar2=-0.5, op0=OP.mult, op1=OP.mult)
                # c0 = -(1 + aa) / (2 sigma^2)
                nc.vector.tensor_scalar(out=tmp, in0=aa, scalar1=1.0,
                                        scalar2=None, op0=OP.add)
                nc.vector.tensor_scalar(out=c0, in0=tmp, scalar1=c1,
                                        scalar2=-0.5, op0=OP.mult, op1=OP.mult)

                nc.sync.dma_start(out=a_out[s0:s0 + psz, :], in_=a_t[:psz])
                nc.sync.dma_start(out=sc_out[s0:s0 + psz, :], in_=sc[:psz, 0:4])
    nc.compile()
    return nc


def _build_lb():
    nc = bacc.Bacc("TRN2", target_bir_lowering=False, debug=False,
                   num_devices=NCORES)
    a_in = nc.declare_dram_parameter("a_in", [HID, SLOTS], BF16,
                                     isOutput=False)
    pool_in = nc.declare_dram_parameter("pool_in", [HID, NG, NCOL], BF16,
                                        isOutput=False)
    scal = nc.declare_dram_parameter("scal", [128, MT, 4], F32,
                                     isOutput=False)
    # scal cols: 0=c0, 1=c1, 2=s, 3=score; pads c0=c1=0, s=-1e4, score=+1e4
    out2 = nc.declare_dram_parameter("out2", [1, 2], F32, isOutput=True)

    with tile.TileContext(nc) as tc:
        with (
            tc.tile_pool(name="singles", bufs=1) as singles,
            tc.tile_pool(name="psum", bufs=7, space="PSUM") as psum_pool,
            tc.tile_pool(name="psum2", bufs=1, space="PSUM") as psum_pool2,
            tc.tile_pool(name="dram", bufs=1, space="DRAM") as dram,
        ):
            engs = (nc.sync, nc.scalar)
            # loads interleaved so matmuls start as soon as data lands:
            # per chunk of 4 M-tiles, the a columns + the 2 pool groups
            a_sb = []
            pool_sb = []
            for kt in range(2):
                a_t = singles.tile([128, SLOTS], BF16, tag=f"a{kt}")
                a_sb.append(a_t)
                p_t = singles.tile([128, NG, NCOL], BF16, tag=f"p{kt}")
                pool_sb.append(p_t)
            NCH = 7
            csz = SLOTS // NCH
            gpc = NG // NCH
            for c in range(NCH):
                for kt in range(2):
                    engs[kt].dma_start(
                        out=a_sb[kt][:, c * csz:(c + 1) * csz],
                        in_=a_in[kt * 128:(kt + 1) * 128,
                                 c * csz:(c + 1) * csz])
                    engs[kt].dma_start(
                        out=pool_sb[kt][:, c * gpc:(c + 1) * gpc, :],
                        in_=pool_in[kt * 128:(kt + 1) * 128,
                                    c * gpc:(c + 1) * gpc, :])
            sc_sb = singles.tile([128, MT, 4], F32, tag="scal")
            nc.gpsimd.dma_start(out=sc_sb, in_=scal[:, :, :])

            t8_sb = singles.tile([128, MT, 8], F32, tag="t8")
            for m in range(MT):
                g = m // (GCAP // 128)
                ps = psum_pool.tile([128, NCOL], F32)
                for kt in range(2):
                    nc.tensor.matmul(
                        out=ps,
                        lhsT=a_sb[kt][:, m * 128:(m + 1) * 128],
                        rhs=pool_sb[kt][:, g, :],
                        start=(kt == 0),
                        stop=(kt == 1),
                    )
                nc.vector.max(out=t8_sb[:, m, :], in_=ps)

            # ---- cross-core exchange of the per-core top-8 --------------
            t8_loc = dram.tile([128, MT, 8], F32)
            t8_all = dram.tile([NCORES, 128, MT, 8], F32)
            nc.gpsimd.dma_start(out=t8_loc[:], in_=t8_sb)
            nc.gpsimd.collective_compute(
                "AllGather",
                OP.bypass,
                replica_groups=[list(range(NCORES))],
                ins=[t8_loc[:].opt()],
                outs=[t8_all[:].opt()],
            )
            t8g = singles.tile([128, NCORES, MT, 8], F32, tag="t8g")
            for c in range(NCORES):
                engs[c % 2].dma_start(out=t8g[:, c, :, :],
                                      in_=t8_all[c, :, :, :])

            # ---- final reduction (replicated on every core) -------------
            t8b = singles.tile([128, MT, 8], F32, tag="t8b")
            for m in range(MT):
                nc.vector.max(out=t8b[:, m, :],
                              in_=t8g[:, :, m, :])
            d8 = singles.tile([128, MT, 8], F32, tag="d8")
            c1b = sc_sb[:, :, 1:2].to_broadcast([128, MT, 8])
            c0b = sc_sb[:, :, 0:1].to_broadcast([128, MT, 8])
            nc.vector.tensor_tensor(out=d8, in0=t8b, in1=c1b, op=OP.mult)
            nc.vector.tensor_tensor(out=d8, in0=d8, in1=c0b, op=OP.add)

            e6 = singles.tile([128, MT, 6], F32, tag="e6")
            nc.scalar.activation(out=e6, in_=d8[:, :, 0:6], func=AF.Exp)
            se6 = singles.tile([128, MT], F32, tag="se6")
            nc.vector.reduce_sum(out=se6[:, :].unsqueeze(-1), in_=e6,
                                 axis=mybir.AxisListType.X)
            numer = singles.tile([128, MT], F32, tag="numer")
            nc.scalar.activation(out=numer[:, :].unsqueeze(-1),
                                 in_=sc_sb[:, :, 2:3], func=AF.Exp)
            mx = singles.tile([128, MT], F32, tag="mx")
            nc.vector.tensor_tensor(out=mx[:, :].unsqueeze(-1),
                                    in0=d8[:, :, 5:6],
                                    in1=sc_sb[:, :, 2:3], op=OP.max)
            em = singles.tile([128, MT], F32, tag="em")
            nc.scalar.activation(out=em, in_=mx, func=AF.Exp)
            deno = singles.tile([128, MT], F32, tag="deno")
            nc.vector.tensor_tensor(out=deno, in0=se6, in1=em, op=OP.subtract)
            nc.vector.tensor_tensor(out=deno, in0=deno, in1=numer, op=OP.add)
            nc.vector.reciprocal(out=deno, in_=deno)
            loss = singles.tile([128, MT], F32, tag="loss")
            nc.vector.tensor_tensor(out=loss, in0=numer, in1=deno, op=OP.mult)

            # softplus(delta - score) = ln(1 + exp(delta - score))
            ones = singles.tile([128, 1], F32, tag="ones")
            nc.vector.memset(ones, 1.0)
            delta_sb = singles.tile([128, 1], F32, tag="delta")
            nc.vector.memset(delta_sb, DELTA)
            spt = singles.tile([128, MT], F32, tag="spt")
            nc.scalar.activation(out=spt[:, :].unsqueeze(-1),
                                 in_=sc_sb[:, :, 3:4], func=AF.Exp,
                                 bias=delta_sb, scale=-1.0)
            sp = singles.tile([128, MT], F32, tag="sp")
            nc.scalar.activation(out=sp, in_=spt, func=AF.Ln, bias=ones)
            red = singles.tile([128, 2], F32, tag="red")
            nc.vector.reduce_sum(out=red[:, 0:1], in_=loss,
                                 axis=mybir.AxisListType.X)
            nc.vector.reduce_sum(out=red[:, 1:2], in_=sp,
                                 axis=mybir.AxisListType.X)
            ps2 = psum_pool2.tile([1, 2], F32)
            nc.tensor.matmul(out=ps2, lhsT=ones[:, 0:1], rhs=red,
                             start=True, stop=True)
            fin = singles.tile([1, 2], F32, tag="fin")
            nc.vector.tensor_scalar(out=fin, in0=ps2, scalar1=1.0 / NSEQ,
                                    scalar2=None, op0=OP.mult)
            nc.sync.dma_start(out=out2[:, :], in_=fin)
    nc.compile()
    return nc


_NC_CACHE = {}


def _get(name, builder):
    if name not in _NC_CACHE:
        _NC_CACHE[name] = builder()
    return _NC_CACHE[name]


LAST_RUNS = []


def _hw_runner(nc, in_maps):
    import os
    res = run_bass_kernel_spmd(
        nc, in_maps, list(range(NCORES)),
        trace=bool(os.environ.get("KERNEL_TRACE")))
    LAST_RUNS.append(res)
    return res.results


def kernel(frame_embeds, other_frame_embeds, W, b, bridge, _runner=None):
    frame_embeds = np.asarray(frame_embeds, dtype=np.float32)
    other_frame_embeds = np.asarray(other_frame_embeds, dtype=np.float32)
    W = np.asarray(W, dtype=np.float32)
    b = np.asarray(b, dtype=np.float32)
    bridge = np.asarray(bridge, dtype=np.int32)

    runner = _runner if _runner is not None else _hw_runner

    # ---- host-side sharding / layout (pure indexing) ----
    fe_seq = frame_embeds.transpose(0, 2, 1, 3).reshape(NSEQ, T, HID)
    ofe_seq = other_frame_embeds.transpose(0, 2, 1, 3).reshape(NSEQ, T, HID)
    perm = np.argsort(bridge[:, 1], kind="stable")
    fe_sorted = fe_seq[perm]
    bridge_s = bridge[perm]

    w_bf = W.astype(NPBF)
    b_bf = np.ascontiguousarray(b.reshape(1, PROJ)).astype(NPBF)
    in_a = []
    for k in range(NCORES):
        sl = slice(k * SPC, (k + 1) * SPC)
        xt = np.zeros((HID, RPAD), dtype=NPBF)
        xt[:, :CUR_ROWS] = (
            fe_sorted[sl].transpose(2, 1, 0).reshape(HID, CUR_ROWS))
        xt[:, CUR_ROWS:ROWS] = (
            ofe_seq[sl, 1:T - 1].transpose(2, 1, 0).reshape(HID, OTH_ROWS))
        br_k = np.ascontiguousarray(bridge_s[sl])
        g1i = ((br_k[:, 1].astype(np.int32) - 1) * SPC
               + np.arange(SPC, dtype=np.int32)).reshape(SPC, 1)
        in_a.append({"xt_in": xt, "w_in": w_bf, "b_in": b_bf,
                     "br_in": br_k, "g1idx": g1i})

    nca = _get("la", _build_la)
    ra = runner(nca, in_a)

    # ---- host reshard between launches (pure indexing) ----
    a_all = np.concatenate([ra[k]["a_out"] for k in range(NCORES)], axis=0)
    sc_all = np.concatenate([ra[k]["sc_out"] for k in range(NCORES)], axis=0)

    piv = bridge_s[:, 1].astype(np.int64)  # sorted ascending, values 1..14
    counts = np.bincount(piv, minlength=T)[1:T - 1]
    assert counts.max() <= GCAP, f"pivot group overflow: {counts.max()}"
    gstart = np.zeros(NG, dtype=np.int64)
    gstart[1:] = np.cumsum(counts)[:-1]
    rank = np.arange(NSEQ, dtype=np.int64) - gstart[piv - 1]
    slot_of = (piv - 1) * GCAP + rank  # slot per sorted row

    a_pad = np.zeros((HID, SLOTS), dtype=NPBF)
    a_pad[:, slot_of] = a_all.T

    scal = np.zeros((128, MT, 4), dtype=np.float32)
    scal[:, :, 2] = -1.0e4
    scal[:, :, 3] = 1.0e4
    scal[slot_of % 128, slot_of // 128, :] = sc_all

    in_b = []
    for k in range(NCORES):
        emb_k = ra[k]["emb"]
        pool_k = np.empty((HID, NG, NCOL), dtype=NPBF)
        pool_k[:, :, :SPC] = (
            emb_k[:POOL_CUR].reshape(NG, SPC, HID).transpose(2, 0, 1))
        pool_k[:, :, SPC:] = (
            emb_k[POOL_CUR:].reshape(NG, SPC, HID).transpose(2, 0, 1))
        in_b.append({"a_in": a_pad, "pool_in": pool_k, "scal": scal})

    ncb = _get("lb", _build_lb)
    rb = runner(ncb, in_b)

    out = rb[0]["out2"]
    brownian_loss = np.float32(out[0, 0])
    head_tail_match = np.float32(out[0, 1])
    return (np.asarray(brownian_loss), np.asarray(head_tail_match))
